# revision 41
# baseline (speedup 1.0000x reference)
"""Trainium2 Bass kernel for sigmoid multi-head attention (B=4, N=2048, C=1024, H=16).

Reference computes (per torch Linear convention, sigmoid attention, no softmax):
  q/k/v = x_{q,k,v} @ W_{q,k,v}.T ; attn = sigmoid(q k^T / sqrt(D)) ;
  out = (attn v) @ Wp.T + bp ; returns (out, attn).

Sharding: 8 cores = 4 batches x 2 head-groups (8 heads each); cores fully
independent (no collectives). Host pre-transposes + pre-casts per-core inputs
to bf16 (x^T [C,N], W^T slices), so the contraction dim is always on SBUF
partitions and the device never transposes anything.

Device dataflow per core (all matmuls full 128x128-tile bf16, f32 PSUM):
  - q^T,k^T projections in transposed orientation, v natural.
  - k^T stored per head zero-padded to 128 partitions (other head's rows 0),
    v stored per head-parity with the other parity's columns zeroed: scores
    and attn@v then run as full-tile matmuls whose zero halves are harmless,
    and a head pair accumulates into shared PSUM banks.
  - scores^T[nk,nq] = kpad_h.T @ qT -> ScalarE sigmoid (scale fused) -> bf16
    attn^T strip -> DMA to DRAM in [h, nk, nq] layout (host un-transposes)
    and -> attn@v accumulation (out^T stacked pair = outz^T).
  - out_part[nq, C] = outz^T.T @ Wp^T-slice, partial per core.
Host: out[b] = part(core 2b) + part(core 2b+1) + bp (f32); attn upcast f32.

Schedule: attention is paced by ScalarE sigmoid (~270us); emission order
stages q/k-mi0 projection first (sigmoids start ~35us in), then the v
projection as a solid block (frees its x slots early so the wave-2 q/k
x-tiles re-loaded from DRAM land in time), attention pairs with attn@v
software-pipelined one iteration behind scores, and q/k projections for
later head pairs placed between pairs to fill PE slack. k-projection PSUM
evacuations run on ScalarE (idle in exactly those windows) so projections
run at full matmul pace instead of DVE-evacuation pace. nq-halved attention
keeps the PSUM footprint at 8 banks (2x2-bank score chunks + 2 attn@v
accumulators + 2 projection rotation). Measured ~403-414 us on-silicon
per NEFF (first correct serial version: 482 us).
"""

import numpy as np
import ml_dtypes

B, N, C, H = 4, 2048, 1024, 16
D = C // H            # 64
HPC = H // 2          # 8 heads per core
CL = HPC * D          # 512 local channels
NCORES = 8
SCALE = D ** -0.5

P = 128
KT = C // P           # 8  k-tiles over c_in
MT = CL // P          # 4  tiles over local channels
NT = N // P           # 16 tiles over sequence
NB = N // 512         # 4  512-wide banks over sequence

_BF16 = ml_dtypes.bfloat16

_CACHED_NC = None


def _build():
    import concourse.mybir as mybir
    import concourse.tile as tile
    from concourse import bacc

    bf16 = mybir.dt.bfloat16
    f32 = mybir.dt.float32
    SIG = mybir.ActivationFunctionType.Sigmoid

    nc = bacc.Bacc("TRN2")

    xqT = nc.declare_dram_parameter("xqT", [C, N], bf16, isOutput=False)
    xkT = nc.declare_dram_parameter("xkT", [C, N], bf16, isOutput=False)
    xvT = nc.declare_dram_parameter("xvT", [C, N], bf16, isOutput=False)
    wqT = nc.declare_dram_parameter("wqT", [C, CL], bf16, isOutput=False)
    wkT = nc.declare_dram_parameter("wkT", [C, CL], bf16, isOutput=False)
    wvT = nc.declare_dram_parameter("wvT", [C, CL], bf16, isOutput=False)
    wpT = nc.declare_dram_parameter("wpT", [CL, C], bf16, isOutput=False)
    attn_out = nc.declare_dram_parameter("attn_out", [HPC, N, N], bf16, isOutput=True)
    out_part = nc.declare_dram_parameter("out_part", [N, C], bf16, isOutput=True)

    with tile.TileContext(nc) as tc:
        with (
            tc.tile_pool(name="big", bufs=16) as pool_big,     # x k-tiles
            tc.tile_pool(name="st", bufs=8) as pool_st,        # attnT half-strips
            tc.tile_pool(name="qt", bufs=MT) as pool_qt,       # qT tiles, live all run
            tc.tile_pool(name="kp", bufs=HPC) as pool_kp,      # padded kT per head
            tc.tile_pool(name="vp", bufs=2 * NT) as pool_v,    # padded v per parity
            tc.tile_pool(name="w", bufs=2 * KT) as pool_w,     # w k-tiles (2 phases live)
            tc.tile_pool(name="wp", bufs=MT) as pool_wp,
            tc.tile_pool(name="oz", bufs=MT) as pool_oz,       # outz^T bf16
            tc.tile_pool(name="ob", bufs=2) as pool_ob,        # final out staging
            tc.tile_pool(name="psA", bufs=4, space="PSUM") as pool_psA,  # 1-bank tiles
            tc.tile_pool(name="psS", bufs=2, space="PSUM") as pool_psS,  # scores 2-bank
        ):
            # ---- padded destination tiles (zero halves written once) ----
            # kpad[hl]: [128, N], rows po..po+64 hold k^T_hl, other rows zero.
            kpad = [
                pool_kp.tile([P, N], bf16, tag="kp", name=f"kpad{hl}")
                for hl in range(HPC)
            ]
            for hl in range(HPC):
                zo = (1 - hl % 2) * D
                nc.vector.memset(kpad[hl][zo:zo + D, :], 0.0)
            # vpad[parity][ni]: [128, CL]; for pair hp, cols hp*128+par*64..+64
            # hold v_{2hp+par}, the other 64 cols of the pair block are zero.
            vpad = [
                [
                    pool_v.tile([P, CL], bf16, tag="v", name=f"vpad{par}_{ni}")
                    for ni in range(NT)
                ]
                for par in range(2)
            ]
            for par in range(2):
                for ni in range(NT):
                    for hp in range(MT):
                        zo = hp * P + (1 - par) * D
                        nc.vector.memset(vpad[par][ni][:, zo:zo + D], 0.0)

            # ---- v projection first (attnV depends on all vpad tiles) ----
            w_tiles = {}
            x_tiles = {}
            qT_tiles = []   # [128, N] x MT  (c_out_local on partitions)

            xv_tiles = []
            for kt in range(KT):
                wt = pool_w.tile([P, CL], bf16, tag="w", name=f"w_v{kt}")
                nc.sync.dma_start(out=wt, in_=wvT[kt * P:(kt + 1) * P, :])
                w_tiles[("v", kt)] = wt
                t = pool_big.tile([P, N], bf16, tag="big", name=f"x_v{kt}")
                nc.sync.dma_start(out=t, in_=xvT[kt * P:(kt + 1) * P, :])
                xv_tiles.append(t)
            for ni in range(NT):
                ps = pool_psA.tile([P, 512], f32, tag="ps", name="vproj_ps")
                for kt in range(KT):
                    nc.tensor.matmul(
                        ps,
                        lhsT=xv_tiles[kt][:, ni * P:(ni + 1) * P],
                        rhs=w_tiles[("v", kt)],
                        start=(kt == 0),
                        stop=(kt == KT - 1),
                    )
                # scatter per head parity into masked v tiles
                for par in range(2):
                    for hp in range(MT):
                        co = hp * P + par * D
                        nc.vector.tensor_copy(
                            out=vpad[par][ni][:, co:co + D], in_=ps[:, co:co + D]
                        )

            # ---- projection helpers ----
            w_tiles = {}
            x_tiles = {}
            qT_tiles = [
                pool_qt.tile([P, N], bf16, tag="qt", name=f"qT{mi}")
                for mi in range(MT)
            ]

            def load_qk_wave(wave, with_v=False):
                tensors = (("q", wqT, xqT), ("k", wkT, xkT)) if not with_v else (
                    ("q", wqT, xqT), ("k", wkT, xkT), ("v", wvT, xvT))
                for kt in range(KT):
                    for name, wdram, dram in tensors:
                        wt = pool_w.tile([P, CL], bf16, tag="w", name=f"w{wave}_{name}{kt}")
                        nc.scalar.dma_start(out=wt, in_=wdram[kt * P:(kt + 1) * P, :])
                        w_tiles[(name, kt)] = wt
                        t = pool_big.tile([P, N], bf16, tag="big", name=f"x{wave}_{name}{kt}")
                        nc.scalar.dma_start(out=t, in_=dram[kt * P:(kt + 1) * P, :])
                        x_tiles[(name, kt)] = t

            def proj_qk_mi(mi):
                for name in ("q", "k"):
                    for nb in range(NB):
                        ps = pool_psA.tile([P, 512], f32, tag="ps", name="proj_ps")
                        for kt in range(KT):
                            nc.tensor.matmul(
                                ps,
                                lhsT=w_tiles[(name, kt)][:, mi * P:(mi + 1) * P],
                                rhs=x_tiles[(name, kt)][:, nb * 512:(nb + 1) * 512],
                                start=(kt == 0),
                                stop=(kt == KT - 1),
                            )
                        if name == "q":
                            nc.vector.tensor_copy(
                                out=qT_tiles[mi][:, nb * 512:(nb + 1) * 512], in_=ps
                            )
                        else:
                            # ScalarE is idle during projection windows (its
                            # sigmoids depend on these); DVE is the pacer here,
                            # so evacuate k-proj PSUM on ScalarE instead.
                            for sub in range(2):
                                hl = mi * 2 + sub
                                po = sub * D
                                nc.scalar.copy(
                                    out=kpad[hl][po:po + D, nb * 512:(nb + 1) * 512],
                                    in_=ps[po:po + D, :],
                                )

            xv_tiles = []

            def load_v():
                for kt in range(KT):
                    wt = pool_w.tile([P, CL], bf16, tag="w", name=f"w_v{kt}")
                    nc.scalar.dma_start(out=wt, in_=wvT[kt * P:(kt + 1) * P, :])
                    w_tiles[("v", kt)] = wt
                    t = pool_big.tile([P, N], bf16, tag="big", name=f"x_v{kt}")
                    nc.scalar.dma_start(out=t, in_=xvT[kt * P:(kt + 1) * P, :])
                    xv_tiles.append(t)

            def proj_v_ni(ni):
                ps = pool_psA.tile([P, 512], f32, tag="ps", name="vproj_ps")
                for kt in range(KT):
                    nc.tensor.matmul(
                        ps,
                        lhsT=xv_tiles[kt][:, ni * P:(ni + 1) * P],
                        rhs=w_tiles[("v", kt)],
                        start=(kt == 0),
                        stop=(kt == KT - 1),
                    )
                for par in range(2):
                    for hp in range(MT):
                        co = hp * P + par * D
                        nc.vector.tensor_copy(
                            out=vpad[par][ni][:, co:co + D], in_=ps[:, co:co + D]
                        )

            # ---- attention pair body (nq-halved for a small PSUM footprint) ----
            outz_tiles = [
                pool_oz.tile([P, N], bf16, tag="oz", name=f"outz_{mi}")
                for mi in range(MT)
            ]
            HF = N // 2  # 1024

            def attention_pair(hp, interleave_vproj=False):
                for half in range(2):
                    avps = [
                        pool_psA.tile([P, 512], f32, tag="ps", name=f"avps_{hp}_{half}_{q2}")
                        for q2 in range(2)
                    ]
                    # software pipeline: attn@v for iteration ni-1 is emitted
                    # between the two scores chunks of iteration ni, so the PE
                    # stream always has ready work while sigmoid runs.
                    pend = []

                    def flush_attnv(items):
                        for pni, pstrips, pq2 in items:
                            for sub in range(2):
                                nc.tensor.matmul(
                                    avps[pq2],
                                    lhsT=vpad[sub][pni][:, hp * P:(hp + 1) * P],
                                    rhs=pstrips[sub][:, pq2 * 512:(pq2 + 1) * 512],
                                    start=(pni == 0 and sub == 0),
                                    stop=(pni == NT - 1 and sub == 1),
                                )

                    for ni in range(NT):
                        if interleave_vproj and half == 0:
                            proj_v_ni(ni)
                        strips = [
                            pool_st.tile([P, HF], bf16, tag="st", name=f"strip{s}")
                            for s in range(2)
                        ]
                        for sub in range(2):
                            sps = pool_psS.tile([P, HF], f32, tag="sps", name=f"sps{sub}")
                            for q2 in range(2):
                                qb = half * 2 + q2
                                nc.tensor.matmul(
                                    sps[:, q2 * 512:(q2 + 1) * 512],
                                    lhsT=kpad[hp * 2 + sub][:, ni * P:(ni + 1) * P],
                                    rhs=qT_tiles[hp][:, qb * 512:(qb + 1) * 512],
                                    start=True,
                                    stop=True,
                                )
                            nc.scalar.activation(
                                strips[sub], sps, SIG, scale=SCALE,
                            )
                            flush_attnv(pend[:1])
                            pend = pend[1:]
                        for sub in range(2):
                            nc.sync.dma_start(
                                out=attn_out[
                                    hp * 2 + sub,
                                    ni * P:(ni + 1) * P,
                                    half * HF:(half + 1) * HF,
                                ],
                                in_=strips[sub],
                            )
                        pend += [(ni, strips, q2) for q2 in range(2)]
                    flush_attnv(pend)
                    for q2 in range(2):
                        nc.vector.tensor_copy(
                            out=outz_tiles[hp][
                                :, half * HF + q2 * 512:half * HF + (q2 + 1) * 512
                            ],
                            in_=avps[q2],
                        )

            def final_proj():
                for mo in range(NT):
                    ob = pool_ob.tile([P, C], bf16, tag="ob", name="ob")
                    for cb in range(2):
                        ps = pool_psA.tile([P, 512], f32, tag="ps", name="fproj_ps")
                        for kt in range(MT):
                            nc.tensor.matmul(
                                ps,
                                lhsT=outz_tiles[kt][:, mo * P:(mo + 1) * P],
                                rhs=wp_tiles[kt][:, cb * 512:(cb + 1) * 512],
                                start=(kt == 0),
                                stop=(kt == MT - 1),
                            )
                        nc.vector.tensor_copy(out=ob[:, cb * 512:(cb + 1) * 512], in_=ps)
                    nc.sync.dma_start(out=out_part[mo * P:(mo + 1) * P, :], in_=ob)

            # ---- orchestration: mi0 projections + v early, attention pairs
            # paced by ScalarE, later-mi projections re-load x from DRAM and
            # fill PE slack during the previous pair's attention phase.
            load_qk_wave(0)
            proj_qk_mi(0)
            load_v()
            wp_tiles = []
            for kt in range(MT):
                wt = pool_wp.tile([P, C], bf16, tag="wp", name=f"wp{kt}")
                nc.scalar.dma_start(out=wt, in_=wpT[kt * P:(kt + 1) * P, :])
                wp_tiles.append(wt)
            for ni in range(NT):
                proj_v_ni(ni)
            attention_pair(0)
            load_qk_wave(1)
            for mi in range(1, MT):
                proj_qk_mi(mi)
                attention_pair(mi)
            final_proj()

    nc.compile()
    return nc


def _get_nc():
    global _CACHED_NC
    if _CACHED_NC is None:
        _CACHED_NC = _build()
    return _CACHED_NC


def _make_in_maps(inputs):
    in_maps = []
    for i in range(NCORES):
        b, g = i // 2, i % 2
        lo, hi = g * CL, (g + 1) * CL
        in_maps.append({
            "xqT": np.asarray(inputs["x_q"][b]).T.astype(_BF16),
            "xkT": np.asarray(inputs["x_k"][b]).T.astype(_BF16),
            "xvT": np.asarray(inputs["x_v"][b]).T.astype(_BF16),
            "wqT": np.asarray(inputs["Wq"])[lo:hi, :].T.astype(_BF16),
            "wkT": np.asarray(inputs["Wk"])[lo:hi, :].T.astype(_BF16),
            "wvT": np.asarray(inputs["Wv"])[lo:hi, :].T.astype(_BF16),
            "wpT": np.asarray(inputs["Wp"])[:, lo:hi].T.astype(_BF16),
        })
    return in_maps


def _assemble(results, inputs):
    out = np.zeros((B, N, C), np.float32)
    attn = np.empty((B, H, N, N), np.float32)
    for i in range(NCORES):
        b, g = i // 2, i % 2
        r = results[i]
        attn[b, g * HPC:(g + 1) * HPC] = (
            np.asarray(r["attn_out"]).astype(np.float32).transpose(0, 2, 1)
        )
        out[b] += np.asarray(r["out_part"]).astype(np.float32)
    out += np.asarray(inputs["bp"]).astype(np.float32)[None, None, :]
    return out, attn


def run(inputs, trace=False, **kwargs):
    from concourse.bass_utils import run_bass_kernel_spmd

    nc = _get_nc()
    in_maps = _make_in_maps(inputs)
    res = run_bass_kernel_spmd(
        nc, in_maps, core_ids=list(range(NCORES)), trace=trace, **kwargs
    )
    out, attn = _assemble(res.results, inputs)
    return (out, attn), res


def kernel(**inputs):
    (out, attn), _ = run(inputs)
    return out, attn


# revision 42
# speedup vs baseline: 1.2225x; 1.2225x over previous
"""Trainium2 Bass kernel for sigmoid multi-head attention (B=4, N=2048, C=1024, H=16).

Reference computes (per torch Linear convention, sigmoid attention, no softmax):
  q/k/v = x_{q,k,v} @ W_{q,k,v}.T ; attn = sigmoid(q k^T / sqrt(D)) ;
  out = (attn v) @ Wp.T + bp ; returns (out, attn).

Sharding: 8 cores = 4 batches x 2 head-groups (8 heads each); cores fully
independent (no collectives). Host pre-transposes + pre-casts per-core inputs
to bf16 (x^T [C,N], W^T slices), so the contraction dim is always on SBUF
partitions and the device never transposes anything.

Device dataflow per core (all matmuls full 128x128-tile bf16, f32 PSUM):
  - q^T,k^T projections in transposed orientation, v natural.
  - k^T stored per head zero-padded to 128 partitions (other head's rows 0),
    v stored per head-parity with the other parity's columns zeroed: scores
    and attn@v then run as full-tile matmuls whose zero halves are harmless,
    and a head pair accumulates into shared PSUM banks.
  - scores^T[nk,nq] = kpad_h.T @ qT -> ScalarE sigmoid (scale fused) -> bf16
    attn^T strip -> DMA to DRAM in [h, nk, nq] layout (host un-transposes)
    and -> attn@v accumulation (out^T stacked pair = outz^T).
  - out_part[nq, C] = outz^T.T @ Wp^T-slice, partial per core.
Host: out[b] = part(core 2b) + part(core 2b+1) + bp (f32); attn upcast f32.

Schedule: attention is paced by ScalarE sigmoid (~270us); emission order
stages q/k-mi0 projection first (sigmoids start ~35us in), then the v
projection as a solid block (frees its x slots early so the wave-2 q/k
x-tiles re-loaded from DRAM land in time), attention pairs with attn@v
software-pipelined one iteration behind scores, and q/k projections for
later head pairs placed between pairs to fill PE slack. k-projection PSUM
evacuations run on ScalarE (idle in exactly those windows) so projections
run at full matmul pace instead of DVE-evacuation pace. nq-halved attention
keeps the PSUM footprint at 8 banks (2x2-bank score chunks + 2 attn@v
accumulators + 2 projection rotation). Measured ~403-414 us on-silicon
per NEFF (first correct serial version: 482 us).
"""

import numpy as np
import ml_dtypes

B, N, C, H = 4, 2048, 1024, 16
D = C // H            # 64
HPC = H // 2          # 8 heads per core
CL = HPC * D          # 512 local channels
NCORES = 8
SCALE = D ** -0.5

P = 128
KT = C // P           # 8  k-tiles over c_in
MT = CL // P          # 4  tiles over local channels
NT = N // P           # 16 tiles over sequence
NB = N // 512         # 4  512-wide banks over sequence

_BF16 = ml_dtypes.bfloat16

_CACHED_NC = None


def _build():
    import concourse.mybir as mybir
    import concourse.tile as tile
    from concourse import bacc

    bf16 = mybir.dt.bfloat16
    f32 = mybir.dt.float32
    SIG = mybir.ActivationFunctionType.Sigmoid

    nc = bacc.Bacc("TRN2")

    xqT = nc.declare_dram_parameter("xqT", [C, N], bf16, isOutput=False)
    xkT = nc.declare_dram_parameter("xkT", [C, N], bf16, isOutput=False)
    xvT = nc.declare_dram_parameter("xvT", [C, N], bf16, isOutput=False)
    wqT = nc.declare_dram_parameter("wqT", [C, CL], bf16, isOutput=False)
    wkT = nc.declare_dram_parameter("wkT", [C, CL], bf16, isOutput=False)
    wvT = nc.declare_dram_parameter("wvT", [C, CL], bf16, isOutput=False)
    wpT = nc.declare_dram_parameter("wpT", [CL, C], bf16, isOutput=False)
    attn_out = nc.declare_dram_parameter("attn_out", [HPC, N, N], bf16, isOutput=True)
    out_part = nc.declare_dram_parameter("out_part", [N, C], bf16, isOutput=True)

    with tile.TileContext(nc) as tc:
        with (
            tc.tile_pool(name="big", bufs=16) as pool_big,     # x k-tiles
            tc.tile_pool(name="st", bufs=8) as pool_st,        # attnT half-strips
            tc.tile_pool(name="qt", bufs=MT) as pool_qt,       # qT tiles, live all run
            tc.tile_pool(name="kp", bufs=HPC) as pool_kp,      # padded kT per head
            tc.tile_pool(name="vp", bufs=2 * NT) as pool_v,    # padded v per parity
            tc.tile_pool(name="w", bufs=2 * KT) as pool_w,     # w k-tiles (2 phases live)
            tc.tile_pool(name="wp", bufs=MT) as pool_wp,
            tc.tile_pool(name="oz", bufs=MT) as pool_oz,       # outz^T bf16
            tc.tile_pool(name="ob", bufs=2) as pool_ob,        # final out staging
            tc.tile_pool(name="psA", bufs=4, space="PSUM") as pool_psA,  # 1-bank tiles
            tc.tile_pool(name="psS", bufs=2, space="PSUM") as pool_psS,  # scores 2-bank
        ):
            # ---- padded destination tiles (zero halves written once) ----
            # kpad[hl]: [128, N], rows po..po+64 hold k^T_hl, other rows zero.
            kpad = [
                pool_kp.tile([P, N], bf16, tag="kp", name=f"kpad{hl}")
                for hl in range(HPC)
            ]
            for hl in range(HPC):
                zo = (1 - hl % 2) * D
                nc.vector.memset(kpad[hl][zo:zo + D, :], 0.0)
            # vpad[parity][ni]: [128, CL]; for pair hp, cols hp*128+par*64..+64
            # hold v_{2hp+par}, the other 64 cols of the pair block are zero.
            vpad = [
                [
                    pool_v.tile([P, CL], bf16, tag="v", name=f"vpad{par}_{ni}")
                    for ni in range(NT)
                ]
                for par in range(2)
            ]
            for par in range(2):
                for ni in range(NT):
                    for hp in range(MT):
                        zo = hp * P + (1 - par) * D
                        nc.vector.memset(vpad[par][ni][:, zo:zo + D], 0.0)

            # ---- v projection first (attnV depends on all vpad tiles) ----
            w_tiles = {}
            x_tiles = {}
            qT_tiles = []   # [128, N] x MT  (c_out_local on partitions)

            xv_tiles = []
            for kt in range(KT):
                wt = pool_w.tile([P, CL], bf16, tag="w", name=f"w_v{kt}")
                nc.sync.dma_start(out=wt, in_=wvT[kt * P:(kt + 1) * P, :])
                w_tiles[("v", kt)] = wt
                t = pool_big.tile([P, N], bf16, tag="big", name=f"x_v{kt}")
                nc.sync.dma_start(out=t, in_=xvT[kt * P:(kt + 1) * P, :])
                xv_tiles.append(t)
            for ni in range(NT):
                ps = pool_psA.tile([P, 512], f32, tag="ps", name="vproj_ps")
                for kt in range(KT):
                    nc.tensor.matmul(
                        ps,
                        lhsT=xv_tiles[kt][:, ni * P:(ni + 1) * P],
                        rhs=w_tiles[("v", kt)],
                        start=(kt == 0),
                        stop=(kt == KT - 1),
                    )
                # scatter per head parity into masked v tiles
                for par in range(2):
                    for hp in range(MT):
                        co = hp * P + par * D
                        nc.vector.tensor_copy(
                            out=vpad[par][ni][:, co:co + D], in_=ps[:, co:co + D]
                        )

            # ---- projection helpers ----
            w_tiles = {}
            x_tiles = {}
            qT_tiles = [
                pool_qt.tile([P, N], bf16, tag="qt", name=f"qT{mi}")
                for mi in range(MT)
            ]

            def load_qk_wave(wave, with_v=False):
                tensors = (("q", wqT, xqT), ("k", wkT, xkT)) if not with_v else (
                    ("q", wqT, xqT), ("k", wkT, xkT), ("v", wvT, xvT))
                for kt in range(KT):
                    for name, wdram, dram in tensors:
                        wt = pool_w.tile([P, CL], bf16, tag="w", name=f"w{wave}_{name}{kt}")
                        nc.sync.dma_start(out=wt, in_=wdram[kt * P:(kt + 1) * P, :])
                        w_tiles[(name, kt)] = wt
                        t = pool_big.tile([P, N], bf16, tag="big", name=f"x{wave}_{name}{kt}")
                        nc.sync.dma_start(out=t, in_=dram[kt * P:(kt + 1) * P, :])
                        x_tiles[(name, kt)] = t

            def proj_qk_mi(mi):
                for name in ("q", "k"):
                    for nb in range(NB):
                        ps = pool_psA.tile([P, 512], f32, tag="ps", name="proj_ps")
                        for kt in range(KT):
                            nc.tensor.matmul(
                                ps,
                                lhsT=w_tiles[(name, kt)][:, mi * P:(mi + 1) * P],
                                rhs=x_tiles[(name, kt)][:, nb * 512:(nb + 1) * 512],
                                start=(kt == 0),
                                stop=(kt == KT - 1),
                            )
                        if name == "q":
                            nc.vector.tensor_copy(
                                out=qT_tiles[mi][:, nb * 512:(nb + 1) * 512], in_=ps
                            )
                        else:
                            # ScalarE is idle during projection windows (its
                            # sigmoids depend on these); DVE is the pacer here,
                            # so evacuate k-proj PSUM on ScalarE instead.
                            for sub in range(2):
                                hl = mi * 2 + sub
                                po = sub * D
                                nc.scalar.copy(
                                    out=kpad[hl][po:po + D, nb * 512:(nb + 1) * 512],
                                    in_=ps[po:po + D, :],
                                )

            xv_tiles = []

            def load_v():
                for kt in range(KT):
                    wt = pool_w.tile([P, CL], bf16, tag="w", name=f"w_v{kt}")
                    nc.sync.dma_start(out=wt, in_=wvT[kt * P:(kt + 1) * P, :])
                    w_tiles[("v", kt)] = wt
                    t = pool_big.tile([P, N], bf16, tag="big", name=f"x_v{kt}")
                    nc.sync.dma_start(out=t, in_=xvT[kt * P:(kt + 1) * P, :])
                    xv_tiles.append(t)

            def proj_v_ni(ni):
                ps = pool_psA.tile([P, 512], f32, tag="ps", name="vproj_ps")
                for kt in range(KT):
                    nc.tensor.matmul(
                        ps,
                        lhsT=xv_tiles[kt][:, ni * P:(ni + 1) * P],
                        rhs=w_tiles[("v", kt)],
                        start=(kt == 0),
                        stop=(kt == KT - 1),
                    )
                for par in range(2):
                    for hp in range(MT):
                        co = hp * P + par * D
                        nc.vector.tensor_copy(
                            out=vpad[par][ni][:, co:co + D], in_=ps[:, co:co + D]
                        )

            # ---- attention pair body (nq-halved for a small PSUM footprint) ----
            outz_tiles = [
                pool_oz.tile([P, N], bf16, tag="oz", name=f"outz_{mi}")
                for mi in range(MT)
            ]
            HF = N // 2  # 1024

            def attention_pair(hp, interleave_vproj=False):
                for half in range(2):
                    avps = [
                        pool_psA.tile([P, 512], f32, tag="ps", name=f"avps_{hp}_{half}_{q2}")
                        for q2 in range(2)
                    ]
                    # software pipeline: attn@v for iteration ni-1 is emitted
                    # between the two scores chunks of iteration ni, so the PE
                    # stream always has ready work while sigmoid runs.
                    pend = []

                    def flush_attnv(items):
                        for pni, pstrips, pq2 in items:
                            for sub in range(2):
                                nc.tensor.matmul(
                                    avps[pq2],
                                    lhsT=vpad[sub][pni][:, hp * P:(hp + 1) * P],
                                    rhs=pstrips[sub][:, pq2 * 512:(pq2 + 1) * 512],
                                    start=(pni == 0 and sub == 0),
                                    stop=(pni == NT - 1 and sub == 1),
                                )

                    for ni in range(NT):
                        if interleave_vproj and half == 0:
                            proj_v_ni(ni)
                        strips = [
                            pool_st.tile([P, HF], bf16, tag="st", name=f"strip{s}")
                            for s in range(2)
                        ]
                        for sub in range(2):
                            sps = pool_psS.tile([P, HF], f32, tag="sps", name=f"sps{sub}")
                            for q2 in range(2):
                                qb = half * 2 + q2
                                nc.tensor.matmul(
                                    sps[:, q2 * 512:(q2 + 1) * 512],
                                    lhsT=kpad[hp * 2 + sub][:, ni * P:(ni + 1) * P],
                                    rhs=qT_tiles[hp][:, qb * 512:(qb + 1) * 512],
                                    start=True,
                                    stop=True,
                                )
                            nc.scalar.activation(
                                strips[sub], sps, SIG, scale=SCALE,
                            )
                            flush_attnv(pend[:1])
                            pend = pend[1:]
                        for sub in range(2):
                            nc.sync.dma_start(
                                out=attn_out[
                                    hp * 2 + sub,
                                    ni * P:(ni + 1) * P,
                                    half * HF:(half + 1) * HF,
                                ],
                                in_=strips[sub],
                            )
                        pend += [(ni, strips, q2) for q2 in range(2)]
                    flush_attnv(pend)
                    for q2 in range(2):
                        nc.vector.tensor_copy(
                            out=outz_tiles[hp][
                                :, half * HF + q2 * 512:half * HF + (q2 + 1) * 512
                            ],
                            in_=avps[q2],
                        )

            def final_proj():
                for mo in range(NT):
                    ob = pool_ob.tile([P, C], bf16, tag="ob", name="ob")
                    for cb in range(2):
                        ps = pool_psA.tile([P, 512], f32, tag="ps", name="fproj_ps")
                        for kt in range(MT):
                            nc.tensor.matmul(
                                ps,
                                lhsT=outz_tiles[kt][:, mo * P:(mo + 1) * P],
                                rhs=wp_tiles[kt][:, cb * 512:(cb + 1) * 512],
                                start=(kt == 0),
                                stop=(kt == MT - 1),
                            )
                        nc.vector.tensor_copy(out=ob[:, cb * 512:(cb + 1) * 512], in_=ps)
                    nc.sync.dma_start(out=out_part[mo * P:(mo + 1) * P, :], in_=ob)

            # ---- orchestration: mi0 projections + v early, attention pairs
            # paced by ScalarE, later-mi projections re-load x from DRAM and
            # fill PE slack during the previous pair's attention phase.
            load_qk_wave(0)
            proj_qk_mi(0)
            load_v()
            wp_tiles = []
            for kt in range(MT):
                wt = pool_wp.tile([P, C], bf16, tag="wp", name=f"wp{kt}")
                nc.sync.dma_start(out=wt, in_=wpT[kt * P:(kt + 1) * P, :])
                wp_tiles.append(wt)
            for ni in range(NT):
                proj_v_ni(ni)
            attention_pair(0)
            load_qk_wave(1)
            for mi in range(1, MT):
                proj_qk_mi(mi)
                attention_pair(mi)
            final_proj()

    nc.compile()
    return nc


def _get_nc():
    global _CACHED_NC
    if _CACHED_NC is None:
        _CACHED_NC = _build()
    return _CACHED_NC


def _make_in_maps(inputs):
    in_maps = []
    for i in range(NCORES):
        b, g = i // 2, i % 2
        lo, hi = g * CL, (g + 1) * CL
        in_maps.append({
            "xqT": np.asarray(inputs["x_q"][b]).T.astype(_BF16),
            "xkT": np.asarray(inputs["x_k"][b]).T.astype(_BF16),
            "xvT": np.asarray(inputs["x_v"][b]).T.astype(_BF16),
            "wqT": np.asarray(inputs["Wq"])[lo:hi, :].T.astype(_BF16),
            "wkT": np.asarray(inputs["Wk"])[lo:hi, :].T.astype(_BF16),
            "wvT": np.asarray(inputs["Wv"])[lo:hi, :].T.astype(_BF16),
            "wpT": np.asarray(inputs["Wp"])[:, lo:hi].T.astype(_BF16),
        })
    return in_maps


def _assemble(results, inputs):
    out = np.zeros((B, N, C), np.float32)
    attn = np.empty((B, H, N, N), np.float32)
    for i in range(NCORES):
        b, g = i // 2, i % 2
        r = results[i]
        attn[b, g * HPC:(g + 1) * HPC] = (
            np.asarray(r["attn_out"]).astype(np.float32).transpose(0, 2, 1)
        )
        out[b] += np.asarray(r["out_part"]).astype(np.float32)
    out += np.asarray(inputs["bp"]).astype(np.float32)[None, None, :]
    return out, attn


def run(inputs, trace=False, **kwargs):
    from concourse.bass_utils import run_bass_kernel_spmd

    nc = _get_nc()
    in_maps = _make_in_maps(inputs)
    res = run_bass_kernel_spmd(
        nc, in_maps, core_ids=list(range(NCORES)), trace=trace, **kwargs
    )
    out, attn = _assemble(res.results, inputs)
    return (out, attn), res


def kernel(**inputs):
    (out, attn), _ = run(inputs)
    return out, attn
